# revision 9
# baseline (speedup 1.0000x reference)
"""CRNN (3x conv-bn-relu-pool + GRU w/ teacher forcing + FC) on 8 TRN2 cores.

Sharding: data-parallel over batch B=32 -> 4 examples/core; weights replicated.
Everything fp32 (teacher-forcing threshold feedback makes reduced precision
cascade into prediction flips).
"""
import sys

sys.path.insert(0, "/opt/trn_rl_repo")

import numpy as np
import concourse.bass as bass
import concourse.bacc as bacc
import concourse.mybir as mybir
import concourse.tile as tile
from concourse.bass_utils import run_bass_kernel_spmd

f32 = mybir.dt.float32
AF = mybir.ActivationFunctionType
OP = mybir.AluOpType
AX = mybir.AxisListType

B, T_FULL, M = 32, 1024, 40
CONV, H, C = 128, 256, 10
EPS = 1e-5
POOLS = (5, 4, 2)
NCORES = 8
BL = B // NCORES  # 4
TAPS = [(dt, dm) for dt in range(5) for dm in range(5)]


def build_nc(T, n_ex):
    nc = bacc.Bacc("TRN2", target_bir_lowering=False)

    # ---- DRAM params (per-core) ----
    p1 = nc.dram_tensor("p1", [n_ex, 1, T + 4, 44], f32, kind="ExternalInput")
    w1 = nc.dram_tensor("w1", [25, 128], f32, kind="ExternalInput")
    w2 = nc.dram_tensor("w2", [128, 25, 128], f32, kind="ExternalInput")
    w3 = nc.dram_tensor("w3", [128, 25, 128], f32, kind="ExternalInput")
    ab = nc.dram_tensor("ab", [128, 6], f32, kind="ExternalInput")  # a1 b1 a2 b2 a3 b3
    wix = nc.dram_tensor("wix", [128, 6, 128], f32, kind="ExternalInput")
    wit = nc.dram_tensor("wit", [10, 6, 128], f32, kind="ExternalInput")
    witb = nc.dram_tensor("witb", [128, 6, 10], f32, kind="ExternalInput")
    imask = nc.dram_tensor("imask", [10, 10], f32, kind="ExternalInput")
    whh = nc.dram_tensor("whh", [128, 2, 6, 128], f32, kind="ExternalInput")
    wf = nc.dram_tensor("wf", [128, 2, 10], f32, kind="ExternalInput")
    bct = nc.dram_tensor("bct", [128, 6], f32, kind="ExternalInput")
    bfs = nc.dram_tensor("bfs", [10, 2], f32, kind="ExternalInput")  # [bf, -bf]
    tgt = nc.dram_tensor("tgt", [10, n_ex, T], f32, kind="ExternalInput")
    fm = nc.dram_tensor("fm", [10, n_ex, T], f32, kind="ExternalInput")
    out_d = nc.dram_tensor("out", [10, T, n_ex], f32, kind="ExternalOutput")

    XC = min(128, T)          # conv1 im2col round length
    T2, T3, TG = 64, min(256, T), min(512, T)

    with tile.TileContext(nc) as tc:
        with tc.tile_pool(name="wpool", bufs=1) as wp:
            w1t = wp.tile([25, 128], f32)
            nc.sync.dma_start(w1t[:], w1[:])
            w2t = wp.tile([128, 25, 128], f32)
            nc.sync.dma_start(w2t[:], w2[:])
            w3t = wp.tile([128, 25, 128], f32)
            nc.sync.dma_start(w3t[:], w3[:])
            abt = wp.tile([128, 6], f32)
            nc.sync.dma_start(abt[:], ab[:])
            wixt = wp.tile([128, 6, 128], f32)
            nc.sync.dma_start(wixt[:], wix[:])
            witt = wp.tile([10, 6, 128], f32)
            nc.sync.dma_start(witt[:], wit[:])
            witbt = wp.tile([128, 6, 10], f32)
            nc.sync.dma_start(witbt[:], witb[:])
            imaskt = wp.tile([10, 10], f32)
            nc.sync.dma_start(imaskt[:], imask[:])
            ones10 = wp.tile([10, 128], f32)
            nc.vector.memset(ones10[:], 1.0)
            whht = wp.tile([128, 2, 6, 128], f32)
            nc.sync.dma_start(whht[:], whh[:])
            wft = wp.tile([128, 2, 10], f32)
            nc.sync.dma_start(wft[:], wf[:])
            bctt = wp.tile([128, 6], f32)
            nc.sync.dma_start(bctt[:], bct[:])
            bfst = wp.tile([10, 2], f32)
            nc.sync.dma_start(bfst[:], bfs[:])
            feats = wp.tile([128, n_ex, T], f32)

            # ============ phase 1: convs ============
            with (
                tc.tile_pool(name="im2col", bufs=2) as px,
                tc.tile_pool(name="planes", bufs=1) as py,
                tc.tile_pool(name="ctmp", bufs=3) as pt,
                tc.tile_pool(name="cpsum", bufs=8, space="PSUM") as cps,
            ):
                y1p = py.tile([128, T + 4, 12], f32)
                nc.vector.memset(y1p[:], 0.0)
                y2p = py.tile([128, T + 4, 6], f32)
                nc.vector.memset(y2p[:], 0.0)

                for ex in range(n_ex):
                    # ---- conv1 ----
                    nc.enter_named_scope(f"conv1_{ex}", False)
                    for rnd in range(T // XC):
                        x25 = px.tile([25, XC, 40], f32, tag="x25")
                        for dt in range(5):
                            base = p1[ex, 0, rnd * XC + dt : rnd * XC + dt + XC, 0:40]
                            src = bass.AP(
                                tensor=base.tensor,
                                offset=base.offset,
                                ap=[[1, 5], [44, XC], [1, 40]],
                            )
                            nc.sync.dma_start(x25[5 * dt : 5 * dt + 5, :, :], src)
                        for ck in range(XC // 8):
                            ps = cps.tile([128, 512], f32, tag="cps")
                            nc.tensor.matmul(
                                ps[:, :320],
                                w1t[:],
                                x25[:, ck * 8 : ck * 8 + 8, :],
                                start=True,
                                stop=True,
                            )
                            t1 = pt.tile([128, 8, 8], f32, tag="t1")
                            nc.vector.reduce_max(
                                t1[:],
                                ps[:, :320].rearrange(
                                    "p (t g f) -> p t g f", t=8, g=8, f=5
                                ),
                                axis=AX.X,
                            )
                            t0 = rnd * XC + ck * 8
                            nc.scalar.activation(
                                y1p[:, 2 + t0 : 2 + t0 + 8, 2:10],
                                t1[:],
                                AF.Relu,
                                bias=abt[:, 1:2],
                                scale=abt[:, 0:1],
                            )
                    nc.leave_named_scope(f"conv1_{ex}", None, False)
                    # ---- conv2 ----
                    nc.enter_named_scope(f"conv2_{ex}", False)
                    n2 = T // T2
                    for g0 in range(0, n2, 8):
                        cks = range(g0, min(g0 + 8, n2))
                        pss = {ck: cps.tile([128, 512], f32, tag="cps", name=f"cps{ck}") for ck in cks}
                        for it, (dt, dm) in enumerate(TAPS):
                            for ck in cks:
                                t0 = ck * T2
                                nc.tensor.matmul(
                                    pss[ck][:, : T2 * 8],
                                    w2t[:, it, :],
                                    y1p[:, dt + t0 : dt + t0 + T2, dm : dm + 8],
                                    start=(it == 0),
                                    stop=(it == 24),
                                )
                        for ck in cks:
                            t0 = ck * T2
                            t2t = pt.tile([128, T2, 2], f32, tag="t2")
                            nc.vector.reduce_max(
                                t2t[:],
                                pss[ck][:, : T2 * 8].rearrange(
                                    "p (t g f) -> p t g f", t=T2, g=2, f=4
                                ),
                                axis=AX.X,
                            )
                            nc.scalar.activation(
                                y2p[:, 2 + t0 : 2 + t0 + T2, 2:4],
                                t2t[:],
                                AF.Relu,
                                bias=abt[:, 3:4],
                                scale=abt[:, 2:3],
                            )
                    nc.leave_named_scope(f"conv2_{ex}", None, False)
                    # ---- conv3 ----
                    nc.enter_named_scope(f"conv3_{ex}", False)
                    n3 = T // T3
                    for g0 in range(0, n3, 8):
                        cks = range(g0, min(g0 + 8, n3))
                        pss = {ck: cps.tile([128, 512], f32, tag="cps", name=f"cps{ck}") for ck in cks}
                        for it, (dt, dm) in enumerate(TAPS):
                            for ck in cks:
                                t0 = ck * T3
                                nc.tensor.matmul(
                                    pss[ck][:, : T3 * 2],
                                    w3t[:, it, :],
                                    y2p[:, dt + t0 : dt + t0 + T3, dm : dm + 2],
                                    start=(it == 0),
                                    stop=(it == 24),
                                )
                        for ck in cks:
                            t0 = ck * T3
                            t3t = pt.tile([128, T3], f32, tag="t3")
                            nc.vector.reduce_max(
                                t3t[:],
                                pss[ck][:, : T3 * 2].rearrange(
                                    "p (t f) -> p t f", t=T3, f=2
                                ),
                                axis=AX.X,
                            )
                            nc.scalar.activation(
                                feats[:, ex, t0 : t0 + T3],
                                t3t[:],
                                AF.Relu,
                                bias=abt[:, 5:6],
                                scale=abt[:, 4:5],
                            )

                    nc.leave_named_scope(f"conv3_{ex}", None, False)
            # ============ phase 2: Gx = Wih_x@feats + Wih_tf@(m*tgt)<<1 + bc ====
            with tc.tile_pool(name="gxpool", bufs=1) as gp:
                nc.enter_named_scope("gx", False)
                gx = gp.tile([128, 6, n_ex, T], f32)
                invm = gp.tile([10, n_ex, T], f32)
                with (
                    tc.tile_pool(name="ph2", bufs=2) as p2,
                    tc.tile_pool(name="gpsum", bufs=6, space="PSUM") as gps,
                ):
                    for ex in range(n_ex):
                        for tck in range(T // TG):
                            ts0 = tck * TG
                            fmc = p2.tile([10, TG], f32, tag="fmc")
                            nc.sync.dma_start(fmc[:], fm[:, ex, ts0 : ts0 + TG])
                            nc.vector.tensor_scalar(
                                invm[:, ex, ts0 : ts0 + TG], fmc[:], -1.0, 1.0,
                                OP.mult, OP.add,
                            )
                            # shifted-by-1 masked targets: column j holds m*tgt at t-1
                            mtc = p2.tile([10, TG], f32, tag="mtc")
                            if tck == 0:
                                nc.vector.memset(mtc[:, 0:1], 0.0)
                            tgc = p2.tile([10, TG], f32, tag="tgc")
                            fsc = p2.tile([10, TG], f32, tag="fsc")
                            lo = 1 if tck == 0 else 0
                            nc.sync.dma_start(
                                tgc[:, lo:], tgt[:, ex, ts0 + lo - 1 : ts0 + TG - 1]
                            )
                            nc.sync.dma_start(
                                fsc[:, lo:], fm[:, ex, ts0 + lo - 1 : ts0 + TG - 1]
                            )
                            nc.vector.tensor_mul(mtc[:, lo:], tgc[:, lo:], fsc[:, lo:])
                            for c in range(6):
                                ps = gps.tile([128, TG], f32, tag="gps")
                                nc.tensor.matmul(
                                    ps[:], wixt[:, c, :],
                                    feats[:, ex, ts0 : ts0 + TG],
                                    start=True, stop=False,
                                )
                                nc.tensor.matmul(
                                    ps[:], witt[:, c, :], mtc[:],
                                    start=False, stop=True,
                                )
                                nc.vector.tensor_scalar(
                                    gx[:, c, ex, ts0 : ts0 + TG], ps[:],
                                    bctt[:, c : c + 1], None, OP.add,
                                )

                nc.leave_named_scope("gx", None, False)
                # ============ phase 3: recurrence ============
                nc.enter_named_scope("rec", False)
                with (
                    tc.tile_pool(name="state", bufs=1) as st,
                    tc.tile_pool(name="rec", bufs=2) as rc,
                    tc.tile_pool(name="pmain", bufs=2, space="PSUM") as ppm,
                    tc.tile_pool(name="pn", bufs=2, space="PSUM") as ppn,
                    tc.tile_pool(name="pq", bufs=2, space="PSUM") as ppq,
                    tc.tile_pool(name="pl", bufs=2, space="PSUM") as ppl,
                ):
                    outbuf = st.tile([10, T, n_ex], f32)
                    ht = st.tile([128, 2, n_ex], f32)
                    nc.vector.memset(ht[:], 0.0)

                    pq_prev = None
                    for t in range(T):
                        pm = ppm.tile([128, 4, n_ex], f32, tag="pm")
                        pmn = ppn.tile([128, 2, n_ex], f32, tag="pmn")
                        pl = ppl.tile([10, n_ex], f32, tag="pl")
                        for c in range(4):
                            nc.tensor.matmul(
                                pm[:, c, :], whht[:, 0, c, :], ht[:, 0, :],
                                start=True, stop=False,
                            )
                            nc.tensor.matmul(
                                pm[:, c, :], whht[:, 1, c, :], ht[:, 1, :],
                                start=False, stop=True,
                            )
                        for c in (4, 5):
                            nc.tensor.matmul(
                                pmn[:, c - 4, :], whht[:, 0, c, :], ht[:, 0, :],
                                start=True, stop=False,
                            )
                            nc.tensor.matmul(
                                pmn[:, c - 4, :], whht[:, 1, c, :], ht[:, 1, :],
                                start=False, stop=True,
                            )
                        # input-side pre-activations: gx[t] + Wih_tf @ q_{t-1}
                        if pq_prev is None:
                            gxt2 = gx[:, :, :, t]
                        else:
                            qv = pq_prev[:].rearrange(
                                "p (j b) -> p j b", j=10
                            ).transpose([0, 2, 1]).unsqueeze(1)
                            o1 = rc.tile([128, 6, n_ex, 10], f32, tag="o1")
                            nc.vector.tensor_mul(
                                o1[:],
                                witbt[:, :, None, :].to_broadcast((128, 6, n_ex, 10)),
                                qv.to_broadcast((128, 6, n_ex, 10)),
                            )
                            tfred = rc.tile([128, 6, n_ex], f32, tag="tfred")
                            nc.vector.reduce_sum(tfred[:], o1[:], axis=AX.X)
                            gxt2t = rc.tile([128, 6, n_ex], f32, tag="gxt2")
                            nc.vector.tensor_add(gxt2t[:], tfred[:], gx[:, :, :, t])
                            gxt2 = gxt2t[:]
                        prz = rc.tile([128, 4, n_ex], f32, tag="prz")
                        nc.vector.tensor_add(prz[:], pm[:], gxt2[:, 0:4, :])
                        rz = rc.tile([128, 4, n_ex], f32, tag="rz")
                        nc.scalar.activation(rz[:], prz[:], AF.Sigmoid)
                        rhn = rc.tile([128, 2, n_ex], f32, tag="rhn")
                        nc.vector.scalar_tensor_tensor(
                            rhn[:], pmn[:], 0.0, rz[:, 0:2, :],
                            OP.bypass, OP.mult,
                        )
                        tn = rc.tile([128, 2, n_ex], f32, tag="tn")
                        nc.vector.tensor_add(tn[:], rhn[:], gxt2[:, 4:6, :])
                        nn = rc.tile([128, 2, n_ex], f32, tag="nn")
                        nc.scalar.activation(nn[:], tn[:], AF.Tanh)
                        dd = rc.tile([128, 2, n_ex], f32, tag="dd")
                        nc.vector.tensor_sub(dd[:], ht[:], nn[:])
                        ee = rc.tile([128, 2, n_ex], f32, tag="ee")
                        nc.vector.tensor_mul(ee[:], rz[:, 2:4, :], dd[:])
                        nc.vector.tensor_add(ht[:], nn[:], ee[:])
                        nc.tensor.matmul(
                            pl[:], wft[:, 0, :], ht[:, 0, :], start=True, stop=False
                        )
                        nc.tensor.matmul(
                            pl[:], wft[:, 1, :], ht[:, 1, :], start=False, stop=True
                        )
                        nc.vector.tensor_scalar(
                            outbuf[:, t, :], pl[:], bfst[:, 0:1], None, OP.add
                        )
                        if t < T - 1:
                            qt = rc.tile([10, n_ex], f32, tag="qt")
                            nc.vector.scalar_tensor_tensor(
                                qt[:], pl[:], bfst[:, 1:2], invm[:, :, t],
                                OP.is_gt, OP.mult,
                            )
                            qm = rc.tile([10, 10, n_ex], f32, tag="qm")
                            nc.vector.tensor_mul(
                                qm[:],
                                qt[:, None, :].to_broadcast((10, 10, n_ex)),
                                imaskt[:, :, None].to_broadcast((10, 10, n_ex)),
                            )
                            pq = ppq.tile([128, 10 * n_ex], f32, tag="pq")
                            nc.tensor.matmul(
                                pq[:], ones10[:], qm[:].rearrange("p j b -> p (j b)"),
                                start=True, stop=True,
                            )
                            pq_prev = pq
                    nc.sync.dma_start(out_d[:], outbuf[:])
                nc.leave_named_scope("rec", None, False)

    nc.compile()
    return nc


def host_prep_shared(inputs):
    """Weight-side host prep (replicated to every core)."""
    W1, W2, W3 = inputs["W1"], inputs["W2"], inputs["W3"]
    d = {}
    d["w1"] = np.ascontiguousarray(W1.reshape(128, 25).T)
    d["w2"] = np.ascontiguousarray(W2.transpose(1, 2, 3, 0).reshape(128, 25, 128))
    d["w3"] = np.ascontiguousarray(W3.transpose(1, 2, 3, 0).reshape(128, 25, 128))
    ab = np.zeros((128, 6), np.float32)
    for i in range(1, 4):
        g, be, m, v, b = (inputs[f"g{i}"], inputs[f"be{i}"], inputs[f"m{i}"],
                          inputs[f"v{i}"], inputs[f"b{i}"])
        alpha = g / np.sqrt(v + EPS)
        assert (alpha > 0).all(), "maxpool/affine commute needs alpha > 0"
        ab[:, 2 * (i - 1)] = alpha
        ab[:, 2 * (i - 1) + 1] = b * alpha + be - m * alpha
    d["ab"] = ab
    Wih, Whh, Wf = inputs["Wih"], inputs["Whh"], inputs["Wf"]
    d["wix"] = np.ascontiguousarray(Wih[:, :128].T.reshape(128, 6, 128))
    d["wit"] = np.ascontiguousarray(Wih[:, 128:138].T.reshape(10, 6, 128))
    d["witb"] = np.ascontiguousarray(
        Wih[:, 128:138].reshape(6, 128, 10).transpose(1, 0, 2)
    )
    d["imask"] = np.eye(10, dtype=np.float32)
    d["whh"] = np.ascontiguousarray(
        Whh.T.reshape(2, 128, 6, 128).transpose(1, 0, 2, 3)
    )
    d["wf"] = np.ascontiguousarray(Wf.T.reshape(2, 128, 10).transpose(1, 0, 2))
    d["bct"] = np.ascontiguousarray((inputs["bih"] + inputs["bhh"]).reshape(6, 128).T)
    bf = inputs["bf"]
    d["bfs"] = np.ascontiguousarray(np.stack([bf, -bf], axis=1))
    return {k: np.ascontiguousarray(v, dtype=np.float32) for k, v in d.items()}


def host_prep_core(inputs, exs, T):
    """Per-core input prep for example indices `exs`."""
    n_ex = len(exs)
    feats = np.asarray(inputs["features"])[exs]  # [n_ex, 1, T, M]
    p1 = np.zeros((n_ex, 1, T + 4, 44), np.float32)
    p1[:, :, 2 : T + 2, 2:42] = feats
    tgt = np.asarray(inputs["targets"])[exs]  # [n_ex, T, C]
    tgtT = np.ascontiguousarray(tgt.transpose(2, 0, 1), dtype=np.float32)
    fmb = np.asarray(inputs["force_mask"])[:, exs]  # [T, n_ex] bool
    fmT = np.ascontiguousarray(
        np.broadcast_to(fmb.T[None, :, :], (10, n_ex, T)), dtype=np.float32
    )
    return {"p1": p1, "tgt": tgtT, "fm": fmT}


_nc_cache = {}


def run_cores(inputs, T, n_ex, trace=False, tmpdir=None):
    key = (T, n_ex)
    if key not in _nc_cache:
        _nc_cache[key] = build_nc(T, n_ex)
    nc = _nc_cache[key]
    shared = host_prep_shared(inputs)
    in_maps = []
    for c in range(NCORES):
        exs = list(range(c * n_ex, (c + 1) * n_ex))
        m = dict(shared)
        m.update(host_prep_core(inputs, exs, T))
        in_maps.append(m)
    res = run_bass_kernel_spmd(
        nc, in_maps, list(range(NCORES)), trace=trace, tmpdir=tmpdir
    )
    outs = []
    for c in range(NCORES):
        o = res.results[c]["out"]  # [10, T, n_ex]
        outs.append(np.ascontiguousarray(o.transpose(2, 1, 0)))  # [n_ex, T, 10]
    full = np.concatenate(outs, axis=0).astype(np.float32)
    return full, res


def kernel(**inputs):
    out, _ = run_cores(inputs, T_FULL, BL)
    return out
